# revision 24
# baseline (speedup 1.0000x reference)
"""Masked-attention kernel for 8 TRN2 NeuronCores (batch-parallel sharding).

v3 design (host-assisted layouts, deferred epilogue, PE/Act/DVE balanced):
  - Host pre-transposes Q/K to [D, S] fp16, packs V as [P, NKT, D] fp16
    (partition-major so DMA runs are 2KB), and packs the mask TRANSPOSED
    as fp8e4m3 (0.0 / 1.0) in [S_k, S_q] layout. No device-side casts or
    prep transposes remain.
  - Scores per k-tile are computed transposed (sc[k, q]): stationary K^T
    tile vs the moving Q^T chunk. The mask bias (-240 * m) folds into the
    same PSUM accumulation with fp8 DoubleRow matmuls at 0.5 cycles/row:
    constants [negI || 0] and [0 || negI] select plane 0/1 of a
    [128, 2, 1024] mask tile, so one mask DMA feeds two k-tiles.
  - exp() on Act is the floor (~1.04us per [128,1024] tile).
  - DVE accumulates exp tiles (fp16 2x); 8 tiny PE matmuls vs a ones
    column give per-q denominators; DVE reciprocal; applied after the
    epilogue transpose as a per-partition scalar.
  - PV: V tile stationary, exp output moving, o^T accumulated in PSUM
    with a 2-tile lag so the PE never waits on exp.
  - Epilogue of chunk c is deferred into chunk c+1's first k-iterations
    (den@kt1, PSUM copy on GPSIMD@kt2, transposes@kt3, scales + output
    DMA@kt3) so the PE pipeline never drains at chunk boundaries.
  - Output is written fp16 in a partition-major packed layout
    [qc, p, t, d]; the host unpacks and casts to fp32.
"""

import numpy as np
import ml_dtypes

B, S, D = 16, 2048, 128
NCORES = 8
BP = B // NCORES  # batches per core
P = 128
QC = 1024  # q-chunk (columns of the transposed score tile)
NQC = S // QC
NKT = S // P  # k tiles
NQS = QC // P  # q subtiles per chunk
HKT = NKT // 2  # k tiles per half-load
SCALE = 1.0 / float(np.sqrt(128.0))
MASK_NEG = -240.0
PVLAG = 3
# mask application split: PE DoubleRow pairs take PE_TILES (plane0/plane1
# per pair), DVE post-exp multiply takes DVE_TILES
PE_TILES = [0, 2, 4, 6, 8, 10, 12, 14, 1, 3]
DVE_TILES = [5, 7, 9, 11, 13, 15]
PAIR_OF = {kt: (i // 2, i % 2) for i, kt in enumerate(PE_TILES)}
DIDX_OF = {kt: i for i, kt in enumerate(DVE_TILES)}
NPAIR = len(PE_TILES) // 2
# last chunk runs DVE-masked tiles first so its tail is mult-free
ORDER_LAST = [5, 7, 1, 3, 9, 11, 13, 15, 0, 2, 4, 6, 8, 10, 12, 14]


def _res_key(kt):
    if kt in PAIR_OF:
        return ("mt", PAIR_OF[kt][0])
    return ("nm", DIDX_OF[kt])


def _res_plan(order):
    seen, plan = set(), [[] for _ in range(NKT)]
    for i, kt in enumerate(order):
        k = _res_key(kt)
        if k not in seen:
            seen.add(k)
            plan[i].append(k)
    return plan

_CACHE = {}


def build_nc(loop=True):
    import concourse.mybir as mybir
    import concourse.tile as tile
    from concourse import bacc

    fp16 = mybir.dt.float16
    fp8 = mybir.dt.float8e4

    nc = bacc.Bacc("TRN2", target_bir_lowering=False, debug=False,
                   num_devices=NCORES)

    QTd = nc.dram_tensor("QT", [BP, D, S], fp16, kind="ExternalInput")
    KTd = nc.dram_tensor("KT", [BP, D, S], fp16, kind="ExternalInput")
    Vd = nc.dram_tensor("V", [BP, P, NKT, D], fp16, kind="ExternalInput")
    Md = nc.dram_tensor("MT", [BP, len(PE_TILES) * P, S], fp8,
                        kind="ExternalInput")
    NMd = nc.dram_tensor("NMT", [BP, len(DVE_TILES) * P, S], fp16,
                         kind="ExternalInput")
    if loop:
        Id = nc.dram_tensor("iters", [1, 1], mybir.dt.int32,
                            kind="ExternalInput")
    Od = nc.dram_tensor("out", [BP, NQC, P, NQS, D], fp16,
                        kind="ExternalOutput")

    # DoubleRow mask-bias weights: plane-selecting [negI || 0] / [0 || negI]
    w0_np = np.zeros((P, 2, P), dtype=np.float32)
    w0_np[:, 0, :] = MASK_NEG * np.eye(P, dtype=np.float32)
    w1_np = np.zeros((P, 2, P), dtype=np.float32)
    w1_np[:, 1, :] = MASK_NEG * np.eye(P, dtype=np.float32)
    w0_dram = nc.inline_tensor(w0_np.astype(ml_dtypes.float8_e4m3),
                               name="w0_const")
    w1_dram = nc.inline_tensor(w1_np.astype(ml_dtypes.float8_e4m3),
                               name="w1_const")
    ident_dram = nc.inline_tensor(np.eye(P, dtype=np.float16),
                                  name="ident_const")

    with tile.TileContext(nc) as tc:
        with tc.tile_pool(name="consts", bufs=1) as consts, \
             tc.tile_pool(name="qkv", bufs=1) as qkv, \
             tc.tile_pool(name="qtp", bufs=2) as qtp, \
             tc.tile_pool(name="maskp", bufs=3) as maskp, \
             tc.tile_pool(name="nmp", bufs=4) as nmp, \
             tc.tile_pool(name="pp", bufs=8) as pp, \
             tc.tile_pool(name="accp", bufs=2) as accp, \
             tc.tile_pool(name="outp", bufs=2) as outp, \
             tc.tile_pool(name="spsum", bufs=2, space="PSUM") as spsum, \
             tc.tile_pool(name="opsum", bufs=1, space="PSUM") as opsum, \
             tc.tile_pool(name="tpsum", bufs=1, space="PSUM") as tpsum:

            # w0 first on the HWDGE ring: it gates the first mask matmul
            w0 = consts.tile([P, 2, P], fp8)
            nc.sync.dma_start(out=w0[:, :, :], in_=w0_dram.ap())
            w1 = consts.tile([P, 2, P], fp8)
            nc.sync.dma_start(out=w1[:, :, :], in_=w1_dram.ap())
            ident = consts.tile([P, P], fp16)
            nc.gpsimd.dma_start(out=ident[:, :], in_=ident_dram.ap())
            ones_col = consts.tile([P, 1], fp16)
            nc.vector.memset(ones_col, 1.0)

            # persistent chunk-0 startup tiles: primed here, reloaded at
            # each loop-body end so the next iteration's first exp is
            # gated only by compute (software-pipelined loop seam)
            fp16_ = mybir.dt.float16
            s0, s1 = ORDER_LAST[0], ORDER_LAST[1]
            pk5 = consts.tile([P, P], fp16_)
            pqt = consts.tile([P, QC], fp16_)
            pk7 = consts.tile([P, P], fp16_)
            pnm0 = consts.tile([P, QC], fp16_)
            pnm1 = consts.tile([P, QC], fp16_)
            pmt4 = consts.tile([P, 2, QC], fp8)
            prime = (pk5, pqt, pk7, pnm0, pnm1, pmt4)

            def load_prime():
                nc.scalar.dma_start(
                    out=pk5[:, :], in_=KTd.ap()[0, :, s0 * P:(s0 + 1) * P])
                nc.scalar.dma_start(out=pqt[:, :], in_=QTd.ap()[0, :, 0:QC])
                nc.scalar.dma_start(
                    out=pk7[:, :], in_=KTd.ap()[0, :, s1 * P:(s1 + 1) * P])
                nc.sync.dma_start(
                    out=pnm0[:, :],
                    in_=NMd.ap()[0, DIDX_OF[s0] * P:(DIDX_OF[s0] + 1) * P,
                                 0:QC])
                nc.sync.dma_start(
                    out=pnm1[:, :],
                    in_=NMd.ap()[0, DIDX_OF[s1] * P:(DIDX_OF[s1] + 1) * P,
                                 0:QC])
                j4 = PAIR_OF[ORDER_LAST[2]][0]
                nc.sync.dma_start(
                    out=pmt4[:, :, :],
                    in_=Md.ap()[0, j4 * 2 * P:(j4 + 1) * 2 * P, 0:QC]
                        .rearrange("(two p) q -> p two q", two=2))

            load_prime()

            pools = (qkv, qtp, maskp, nmp, pp, accp, outp, spsum, opsum, tpsum)
            if loop:
                it_sb = consts.tile([1, 1], mybir.dt.int32)
                nc.sync.dma_start(out=it_sb[:, :], in_=Id.ap())
                n_iters = nc.values_load(it_sb[:, :],
                                         skip_runtime_bounds_check=True)
                with tc.For_i(0, n_iters, 1,
                              hint_engines=(mybir.EngineType.PE,
                                            mybir.EngineType.Activation,
                                            mybir.EngineType.DVE,
                                            mybir.EngineType.SP,
                                            mybir.EngineType.Pool)):
                    _kernel_body(nc, mybir, QTd, KTd, Vd, Md, NMd, Od,
                                 w0, w1, ident, ones_col, prime,
                                 load_prime, *pools)
            else:
                _kernel_body(nc, mybir, QTd, KTd, Vd, Md, NMd, Od,
                             w0, w1, ident, ones_col, prime,
                             load_prime, *pools)
    nc.compile()
    return nc


def _kernel_body(nc, mybir, QTd, KTd, Vd, Md, NMd, Od, w0, w1, ident,
                 ones_col, prime, load_prime,
                 qkv, qtp, maskp, nmp, pp, accp, outp, spsum, opsum,
                 tpsum):
    fp16 = mybir.dt.float16
    fp32 = mybir.dt.float32
    fp8 = mybir.dt.float8e4
    Exp = mybir.ActivationFunctionType.Exp

    def load_mask_pair(b, qc, j):
        # [128, 2, 1024] fp8: plane 0 = k-tile 4j, plane 1 = k-tile 4j+2
        # (Md holds only the even k-tiles' mask rows, transposed)
        t = maskp.tile([P, 2, QC], fp8, name="mtile")
        nc.sync.dma_start(
            out=t[:, :, :],
            in_=Md.ap()[b, j * 2 * P:(j + 1) * 2 * P,
                        qc * QC:(qc + 1) * QC]
                .rearrange("(two p) q -> p two q", two=2))
        return t

    def load_nm(b, qc, o):
        # [128, 1024] fp16 not-mask for odd k-tile 2o+1
        t = nmp.tile([P, QC], fp16, name="nmtile")
        nc.sync.dma_start(
            out=t[:, :],
            in_=NMd.ap()[b, o * P:(o + 1) * P, qc * QC:(qc + 1) * QC])
        return t

    def load_qt(b, qc):
        t = qtp.tile([P, QC], fp16, name="qt")
        nc.sync.dma_start(out=t[:, :],
                            in_=QTd.ap()[b, :, qc * QC:(qc + 1) * QC])
        return t

    def load_k_half(b, h):
        t = qkv.tile([P, HKT * P], fp16, name=f"kt{b}{h}")
        nc.sync.dma_start(
            out=t[:, :], in_=KTd.ap()[b, :, h * HKT * P:(h + 1) * HKT * P])
        return t

    def load_v_half(b, h):
        t = qkv.tile([P, HKT, D], fp16, name=f"v{b}{h}")
        nc.sync.dma_start(
            out=t[:, :, :], in_=Vd.ap()[b, :, h * HKT:(h + 1) * HKT, :])
        return t

    mt_next = {}
    nm_next = {}

    def get_mt(b, qc, j):
        t = mt_next.get((b, qc, j))
        if t is None:
            t = mt_next[(b, qc, j)] = load_mask_pair(b, qc, j)
        return t

    def get_nm(b, qc, o):
        t = nm_next.get((b, qc, o))
        if t is None:
            t = nm_next[(b, qc, o)] = load_nm(b, qc, o)
        return t

    def get_res(b, qc, key):
        if key[0] == "mt":
            get_mt(b, qc, key[1])
        else:
            get_nm(b, qc, key[1])

    # chunk-0 startup resources come from the primed persistent tiles
    pk5, pqt, pk7, pnm0, pnm1, pmt4 = prime
    s0, s1 = ORDER_LAST[0], ORDER_LAST[1]
    kt0x = {s0: pk5, s1: pk7}
    qt_next = {(0, 0): pqt}
    nm_next[(0, 0, DIDX_OF[s0])] = pnm0
    nm_next[(0, 0, DIDX_OF[s1])] = pnm1
    mt_next[(0, 0, PAIR_OF[ORDER_LAST[2]][0])] = pmt4
    kv = {0: [load_k_half(0, 0), load_v_half(0, 0),
              load_k_half(0, 1), load_v_half(0, 1)]}

    pend = {}
    pend_pv = []

    def emit_pv(ops, pts, jj, vh0, vh1, first, final):
        vsel = vh0 if jj < HKT else vh1
        for n in range(0, QC, 512):
            nc.tensor.matmul(
                ops[:, n:n + 512],
                lhsT=vsel[:, jj % HKT, :],
                rhs=pts[jj][:, n:n + 512],
                start=first, stop=final,
                skip_group_check=True)
        del pts[jj]

    def epi_den(c):
        acc, _, b, qc = pend[c]
        den = tpsum.tile([P, NQS], fp32, name="den")
        for sq in range(NQS):
            nc.tensor.matmul(den[:, sq:sq + 1],
                             lhsT=acc[:, sq * P:(sq + 1) * P],
                             rhs=ones_col[:, :],
                             start=True, stop=True,
                             skip_group_check=True)
        rcol = outp.tile([P, NQS], fp32, name="rcol")
        nc.vector.reciprocal(out=rcol[:, :], in_=den[:, :])
        pend[c] += (rcol,)

    def epi_copy(c, last=False):
        _, ops, b, qc, _ = pend[c]
        # PSUM drain on DVE (GPSIMD cannot access PSUM on real HW); in
        # the final flush Act is already idle, so it takes half
        ot = outp.tile([P, QC], fp16, name="ot")
        if last:
            H = QC // 2
            nc.vector.tensor_copy(out=ot[:, :H], in_=ops[:, :H])
            nc.scalar.copy(out=ot[:, H:], in_=ops[:, H:])
        else:
            nc.vector.tensor_copy(out=ot[:, :], in_=ops[:, :])
        pend[c] += (ot,)

    def epi_out(c, last=False):
        _, _, b, qc, rcol, ot = pend.pop(c)
        osb = tpsum.tile([P, QC], fp16, name="osb")
        osf = outp.tile([P, NQS, D], fp16, name="osf")
        HQ = NQS // 2
        for hh in range(2):
            for t in range(hh * HQ, (hh + 1) * HQ):
                nc.tensor.transpose(osb[:, t * P:(t + 1) * P],
                                    ot[:, t * P:(t + 1) * P],
                                    ident[:, :])
            for t in range(hh * HQ, (hh + 1) * HQ):
                if last and t % 2 == 1:
                    nc.scalar.activation(
                        out=osf[:, t, :],
                        in_=osb[:, t * P:(t + 1) * P],
                        func=mybir.ActivationFunctionType.Copy,
                        scale=rcol[:, t:t + 1])
                else:
                    nc.vector.tensor_scalar_mul(
                        out=osf[:, t, :],
                        in0=osb[:, t * P:(t + 1) * P],
                        scalar1=rcol[:, t:t + 1])
            ring = (nc.sync.dma_start if (hh == 0 or last)
                    else nc.gpsimd.dma_start)
            ring(out=Od.ap()[b, qc, :, hh * HQ:(hh + 1) * HQ, :],
                 in_=osf[:, hh * HQ:(hh + 1) * HQ, :])

    for b in range(BP):
        for qc in range(NQC):
            c = b * NQC + qc
            kh0, vh0, kh1, vh1 = kv[b]
            qt = qt_next.pop((b, qc))
            if qc + 1 < NQC:
                nb, nqc = b, qc + 1
            elif b + 1 < BP:
                nb, nqc = b + 1, 0
            else:
                nb = None
            acc = accp.tile([P, QC], fp16, name="acc")
            ops = opsum.tile([P, QC], fp32, name="opsum")
            pts = {}
            last = nb is None
            order = ORDER_LAST if (last or c == 0) else list(range(NKT))
            res_plan = _res_plan(order)
            if nb is not None:
                next_last = (nb == BP - 1 and nqc == NQC - 1)
                next_plan = _res_plan(
                    ORDER_LAST if next_last else range(NKT))
            for i, kt in enumerate(order):
                sc = spsum.tile([P, QC], fp32, name="scores")
                if kt in PAIR_OF:
                    j, plane = PAIR_OF[kt]
                    mt = get_mt(b, qc, j)
                    wsel = w0 if plane == 0 else w1
                    for n in range(0, QC, 512):
                        nc.tensor.matmul(
                            sc[:, n:n + 512],
                            lhsT=wsel[:, :, :],
                            rhs=mt[:, :, n:n + 512],
                            start=True, stop=False,
                            perf_mode=mybir.MatmulPerfMode.DoubleRow,
                            skip_group_check=True)
                    qk_start = False
                else:
                    nm = get_nm(b, qc, DIDX_OF[kt])
                    qk_start = True
                if c == 0 and kt in kt0x and i < 2:
                    ksel, kloc = kt0x[kt], 0
                else:
                    ksel = kh0 if kt < HKT else kh1
                    kloc = (kt % HKT) * P
                for n in range(0, QC, 512):
                    nc.tensor.matmul(
                        sc[:, n:n + 512],
                        lhsT=ksel[:, kloc:kloc + P],
                        rhs=qt[:, n:n + 512],
                        start=qk_start, stop=True, skip_group_check=True)

                # previous chunk's PV tail + deferred epilogue, placed
                # AFTER this kt's QK so the Act pipeline never bubbles;
                # epi_copy precedes PV(c, 0) (single-buffered ops PSUM)
                if pend_pv:
                    if i == 0:
                        emit_pv(*pend_pv.pop(0))
                        emit_pv(*pend_pv.pop(0))
                    elif i == 1:
                        emit_pv(*pend_pv.pop(0))
                if c - 1 in pend:
                    if i == 1:
                        epi_den(c - 1)
                    elif i == 2:
                        epi_copy(c - 1)
                    elif i == 4:
                        epi_out(c - 1)

                pt = pp.tile([P, QC], fp16, name="pt")
                nc.scalar.activation(out=pt[:, :], in_=sc[:, :],
                                     func=Exp, scale=SCALE)
                if kt in PAIR_OF:
                    pts[kt] = pt
                else:
                    # apply the mask as an exact post-exp zeroing on DVE
                    pm = pts[kt] = pp.tile([P, QC], fp16, name="pm")
                    nc.vector.tensor_mul(out=pm[:, :], in0=pt[:, :],
                                         in1=nm[:, :])
                if i == 1:
                    nc.vector.tensor_add(out=acc[:, :],
                                         in0=pts[order[0]][:, :],
                                         in1=pts[order[1]][:, :])
                elif i > 1:
                    nc.vector.tensor_add(out=acc[:, :], in0=acc[:, :],
                                         in1=pts[kt][:, :])

                # prefetches (after compute emission so they never gate
                # it): stay ~3 k-tiles ahead in resource consumption order
                if i + 3 < NKT:
                    for key in res_plan[i + 3]:
                        get_res(b, qc, key)
                if i >= NKT - 3 and nb is not None:
                    for key in next_plan[i - (NKT - 3)]:
                        get_res(nb, nqc, key)
                if i == 6 and nb is not None:
                    qt_next[(nb, nqc)] = load_qt(nb, nqc)
                if nb is not None and nqc == 0:
                    if i == 8:
                        kv[nb] = [load_k_half(nb, 0), load_v_half(nb, 0)]
                    elif i == 10:
                        kv[nb] += [load_k_half(nb, 1), load_v_half(nb, 1)]

                # PV lags PVLAG k-tiles so the PE never waits on exp
                if i >= PVLAG:
                    emit_pv(ops, pts, order[i - PVLAG], vh0, vh1,
                            first=(i == PVLAG), final=False)
            tail = order[NKT - PVLAG:]
            for x, jj in enumerate(tail):
                pend_pv.append((ops, pts, jj, vh0, vh1, False,
                                x == len(tail) - 1))
            pend[c] = (acc, ops, b, qc)

    # final flush (no next chunk to hide it in)
    while pend_pv:
        emit_pv(*pend_pv.pop(0))
    c = BP * NQC - 1
    epi_den(c)
    epi_copy(c, last=True)
    epi_out(c, last=True)
    # software-pipelined seam: refill the chunk-0 startup tiles so the
    # next loop iteration starts compute immediately
    load_prime()


def _get_nc(loop=False):
    key = f"nc_loop{loop}"
    if key not in _CACHE:
        _CACHE[key] = build_nc(loop=loop)
    return _CACHE[key]


def make_in_maps(Q, K, V, mask):
    """Host-side shard + layout prep: per-core input dicts."""
    Q = np.asarray(Q, dtype=np.float32)
    K = np.asarray(K, dtype=np.float32)
    V = np.asarray(V, dtype=np.float32)
    mask_b = np.asarray(mask).astype(bool)
    in_maps = []
    for c in range(NCORES):
        sl = slice(c * BP, (c + 1) * BP)
        qt = np.ascontiguousarray(
            Q[sl].transpose(0, 2, 1)).astype(np.float16)
        kt = np.ascontiguousarray(
            K[sl].transpose(0, 2, 1)).astype(np.float16)
        # V packed partition-major: [BP, P, NKT, D]
        v16 = np.ascontiguousarray(
            V[sl].reshape(BP, NKT, P, D).transpose(0, 2, 1, 3)
        ).astype(np.float16)
        mT = np.ascontiguousarray(mask_b[sl].transpose(0, 2, 1))
        mT4 = mT.reshape(BP, NKT, P, S)
        mt8 = np.ascontiguousarray(mT4[:, PE_TILES]).reshape(
            BP, len(PE_TILES) * P, S).astype(ml_dtypes.float8_e4m3)
        nmt = np.ascontiguousarray(~mT4[:, DVE_TILES]).reshape(
            BP, len(DVE_TILES) * P, S).astype(np.float16)
        in_maps.append({"QT": qt, "KT": kt, "V": v16, "MT": mt8,
                        "NMT": nmt})
    return in_maps


def unpack_out(raw):
    """[BP, NQC, P, NQS, D] fp16 -> [BP, S, D] fp32."""
    return np.ascontiguousarray(
        raw.transpose(0, 1, 3, 2, 4)).reshape(BP, S, D).astype(np.float32)


def kernel(Q, K, V, mask, dk=128):
    from concourse.bass_utils import run_bass_kernel_spmd

    assert int(dk) == 128
    nc = _get_nc(loop=False)
    in_maps = make_in_maps(Q, K, V, mask)
    res = run_bass_kernel_spmd(nc, in_maps, core_ids=list(range(NCORES)))
    return np.concatenate([unpack_out(r["out"]) for r in res.results],
                          axis=0)


# revision 25
# speedup vs baseline: 1.3052x; 1.3052x over previous
"""Masked-attention kernel for 8 TRN2 NeuronCores (batch-parallel sharding).

Per-core shard: 2 batches of [S=2048, D=128] Q/K/V + [S, S] bool mask.

Design (host-assisted layouts; PE / Act / DVE balanced against the
Activation-engine exp floor of ~1.04us per [128, 1024] tile):
  - Host pre-transposes Q/K to [D, S] fp16, packs V partition-major
    [P, NKT, D] fp16 (2KB DMA runs), and splits the transposed mask two
    ways: fp8e4m3 0/1 planes for the 10 PE-handled k-tiles and inverted
    fp16 not-masks for the 6 DVE-handled k-tiles. No device-side casts
    or layout transposes remain.
  - Scores per k-tile are computed transposed (sc[k, q] = K_tile^T Q^T):
    stationary K^T tile, moving Q^T chunk, fp16 at 1 col/cycle.
  - Mask application, PE path: the -240*m bias folds into the score
    accumulation with fp8 DoubleRow matmuls at 0.5 cycles/row; constants
    [negI || 0] / [0 || negI] select plane 0/1 of a [128, 2, 1024] mask
    tile so one mask DMA feeds two k-tiles.
  - Mask application, DVE path: exact post-exp zeroing pm = pt * notmask
    (fp16 2x mode), balancing PE against the Act floor.
  - Softmax denominator: DVE accumulates exp tiles; 8 one-column PE
    matmuls vs a ones vector give per-q denominators; DVE reciprocal,
    applied per-partition after the epilogue transpose.
  - PV: V tile stationary, masked exp output moving, o^T accumulated in
    PSUM across k-tiles with a 3-tile lag.
  - Chunk epilogues (denominator, PSUM drain, transposes, scales, store)
    are deferred into the next chunk's first k-iterations, and the PV
    tail slides after the next chunk's first QKs, so neither PE nor Act
    ever drains at a chunk boundary.
  - The last chunk runs its DVE-masked tiles first (ORDER_LAST) so the
    final serial tail has no DVE mask-mult in it, and the final flush
    borrows the then-idle Act engine for half the drain and scales.
  - The loop (timing) build software-pipelines the iteration seam:
    chunk-0 startup tiles are persistent, primed pre-loop and refilled
    at body end, so the next iteration's first exp is compute-gated.
  - Output is written fp16 in a packed [qc, p, t, d] layout; the host
    unpacks to [S, D] and casts to fp32.
"""

import numpy as np
import ml_dtypes

B, S, D = 16, 2048, 128
NCORES = 8
BP = B // NCORES  # batches per core
P = 128
QC = 1024  # q-chunk (columns of the transposed score tile)
NQC = S // QC
NKT = S // P  # k tiles
NQS = QC // P  # q subtiles per chunk
HKT = NKT // 2  # k tiles per half-load
SCALE = 1.0 / float(np.sqrt(128.0))
MASK_NEG = -240.0
PVLAG = 3
# mask application split: PE DoubleRow pairs take PE_TILES (plane0/plane1
# per pair), DVE post-exp multiply takes DVE_TILES
PE_TILES = [0, 2, 4, 6, 8, 10, 12, 14, 1, 3]
DVE_TILES = [5, 7, 9, 11, 13, 15]
PAIR_OF = {kt: (i // 2, i % 2) for i, kt in enumerate(PE_TILES)}
DIDX_OF = {kt: i for i, kt in enumerate(DVE_TILES)}
NPAIR = len(PE_TILES) // 2
# last chunk runs DVE-masked tiles first so its tail is mult-free
ORDER_LAST = [5, 7, 1, 3, 9, 11, 13, 15, 0, 2, 4, 6, 8, 10, 12, 14]


def _res_key(kt):
    if kt in PAIR_OF:
        return ("mt", PAIR_OF[kt][0])
    return ("nm", DIDX_OF[kt])


def _res_plan(order):
    seen, plan = set(), [[] for _ in range(NKT)]
    for i, kt in enumerate(order):
        k = _res_key(kt)
        if k not in seen:
            seen.add(k)
            plan[i].append(k)
    return plan

_CACHE = {}


def build_nc(loop=True):
    import concourse.mybir as mybir
    import concourse.tile as tile
    from concourse import bacc

    fp16 = mybir.dt.float16
    fp8 = mybir.dt.float8e4

    nc = bacc.Bacc("TRN2", target_bir_lowering=False, debug=False,
                   num_devices=NCORES)

    QTd = nc.dram_tensor("QT", [BP, D, S], fp16, kind="ExternalInput")
    KTd = nc.dram_tensor("KT", [BP, D, S], fp16, kind="ExternalInput")
    Vd = nc.dram_tensor("V", [BP, P, NKT, D], fp16, kind="ExternalInput")
    Md = nc.dram_tensor("MT", [BP, len(PE_TILES) * P, S], fp8,
                        kind="ExternalInput")
    NMd = nc.dram_tensor("NMT", [BP, len(DVE_TILES) * P, S], fp16,
                         kind="ExternalInput")
    if loop:
        Id = nc.dram_tensor("iters", [1, 1], mybir.dt.int32,
                            kind="ExternalInput")
    Od = nc.dram_tensor("out", [BP, NQC, P, NQS, D], fp16,
                        kind="ExternalOutput")

    # DoubleRow mask-bias weights: plane-selecting [negI || 0] / [0 || negI]
    w0_np = np.zeros((P, 2, P), dtype=np.float32)
    w0_np[:, 0, :] = MASK_NEG * np.eye(P, dtype=np.float32)
    w1_np = np.zeros((P, 2, P), dtype=np.float32)
    w1_np[:, 1, :] = MASK_NEG * np.eye(P, dtype=np.float32)
    w0_dram = nc.inline_tensor(w0_np.astype(ml_dtypes.float8_e4m3),
                               name="w0_const")
    w1_dram = nc.inline_tensor(w1_np.astype(ml_dtypes.float8_e4m3),
                               name="w1_const")
    ident_dram = nc.inline_tensor(np.eye(P, dtype=np.float16),
                                  name="ident_const")

    with tile.TileContext(nc) as tc:
        with tc.tile_pool(name="consts", bufs=1) as consts, \
             tc.tile_pool(name="qkv", bufs=1) as qkv, \
             tc.tile_pool(name="qtp", bufs=2) as qtp, \
             tc.tile_pool(name="maskp", bufs=3) as maskp, \
             tc.tile_pool(name="nmp", bufs=4) as nmp, \
             tc.tile_pool(name="pp", bufs=8) as pp, \
             tc.tile_pool(name="accp", bufs=2) as accp, \
             tc.tile_pool(name="outp", bufs=2) as outp, \
             tc.tile_pool(name="spsum", bufs=2, space="PSUM") as spsum, \
             tc.tile_pool(name="opsum", bufs=1, space="PSUM") as opsum, \
             tc.tile_pool(name="tpsum", bufs=1, space="PSUM") as tpsum:

            # w0 first on the HWDGE ring: it gates the first mask matmul
            w0 = consts.tile([P, 2, P], fp8)
            nc.sync.dma_start(out=w0[:, :, :], in_=w0_dram.ap())
            w1 = consts.tile([P, 2, P], fp8)
            nc.sync.dma_start(out=w1[:, :, :], in_=w1_dram.ap())
            ident = consts.tile([P, P], fp16)
            nc.gpsimd.dma_start(out=ident[:, :], in_=ident_dram.ap())
            ones_col = consts.tile([P, 1], fp16)
            nc.vector.memset(ones_col, 1.0)

            # persistent chunk-0 startup tiles: primed here, reloaded at
            # each loop-body end so the next iteration's first exp is
            # gated only by compute (software-pipelined loop seam)
            fp16_ = mybir.dt.float16
            s0, s1 = ORDER_LAST[0], ORDER_LAST[1]
            pk5 = consts.tile([P, P], fp16_)
            pqt = consts.tile([P, QC], fp16_)
            pk7 = consts.tile([P, P], fp16_)
            pnm0 = consts.tile([P, QC], fp16_)
            pnm1 = consts.tile([P, QC], fp16_)
            pmt4 = consts.tile([P, 2, QC], fp8)
            prime = (pk5, pqt, pk7, pnm0, pnm1, pmt4)

            def load_prime():
                nc.scalar.dma_start(
                    out=pk5[:, :], in_=KTd.ap()[0, :, s0 * P:(s0 + 1) * P])
                nc.scalar.dma_start(out=pqt[:, :], in_=QTd.ap()[0, :, 0:QC])
                nc.scalar.dma_start(
                    out=pk7[:, :], in_=KTd.ap()[0, :, s1 * P:(s1 + 1) * P])
                nc.sync.dma_start(
                    out=pnm0[:, :],
                    in_=NMd.ap()[0, DIDX_OF[s0] * P:(DIDX_OF[s0] + 1) * P,
                                 0:QC])
                nc.sync.dma_start(
                    out=pnm1[:, :],
                    in_=NMd.ap()[0, DIDX_OF[s1] * P:(DIDX_OF[s1] + 1) * P,
                                 0:QC])
                j4 = PAIR_OF[ORDER_LAST[2]][0]
                nc.sync.dma_start(
                    out=pmt4[:, :, :],
                    in_=Md.ap()[0, j4 * 2 * P:(j4 + 1) * 2 * P, 0:QC]
                        .rearrange("(two p) q -> p two q", two=2))

            load_prime()

            pools = (qkv, qtp, maskp, nmp, pp, accp, outp, spsum, opsum, tpsum)
            if loop:
                it_sb = consts.tile([1, 1], mybir.dt.int32)
                nc.sync.dma_start(out=it_sb[:, :], in_=Id.ap())
                n_iters = nc.values_load(it_sb[:, :],
                                         skip_runtime_bounds_check=True)
                with tc.For_i(0, n_iters, 1,
                              hint_engines=(mybir.EngineType.PE,
                                            mybir.EngineType.Activation,
                                            mybir.EngineType.DVE,
                                            mybir.EngineType.SP,
                                            mybir.EngineType.Pool)):
                    _kernel_body(nc, mybir, QTd, KTd, Vd, Md, NMd, Od,
                                 w0, w1, ident, ones_col, prime,
                                 load_prime, *pools)
            else:
                _kernel_body(nc, mybir, QTd, KTd, Vd, Md, NMd, Od,
                             w0, w1, ident, ones_col, prime,
                             None, *pools)
    nc.compile()
    return nc


def _kernel_body(nc, mybir, QTd, KTd, Vd, Md, NMd, Od, w0, w1, ident,
                 ones_col, prime, load_prime,
                 qkv, qtp, maskp, nmp, pp, accp, outp, spsum, opsum,
                 tpsum):
    fp16 = mybir.dt.float16
    fp32 = mybir.dt.float32
    fp8 = mybir.dt.float8e4
    Exp = mybir.ActivationFunctionType.Exp

    def load_mask_pair(b, qc, j):
        # [128, 2, 1024] fp8: plane 0 = k-tile 4j, plane 1 = k-tile 4j+2
        # (Md holds only the even k-tiles' mask rows, transposed)
        t = maskp.tile([P, 2, QC], fp8, name="mtile")
        nc.sync.dma_start(
            out=t[:, :, :],
            in_=Md.ap()[b, j * 2 * P:(j + 1) * 2 * P,
                        qc * QC:(qc + 1) * QC]
                .rearrange("(two p) q -> p two q", two=2))
        return t

    def load_nm(b, qc, o):
        # [128, 1024] fp16 not-mask for odd k-tile 2o+1
        t = nmp.tile([P, QC], fp16, name="nmtile")
        nc.sync.dma_start(
            out=t[:, :],
            in_=NMd.ap()[b, o * P:(o + 1) * P, qc * QC:(qc + 1) * QC])
        return t

    def load_qt(b, qc):
        t = qtp.tile([P, QC], fp16, name="qt")
        nc.sync.dma_start(out=t[:, :],
                            in_=QTd.ap()[b, :, qc * QC:(qc + 1) * QC])
        return t

    def load_k_half(b, h):
        t = qkv.tile([P, HKT * P], fp16, name=f"kt{b}{h}")
        nc.sync.dma_start(
            out=t[:, :], in_=KTd.ap()[b, :, h * HKT * P:(h + 1) * HKT * P])
        return t

    def load_v_half(b, h):
        t = qkv.tile([P, HKT, D], fp16, name=f"v{b}{h}")
        nc.sync.dma_start(
            out=t[:, :, :], in_=Vd.ap()[b, :, h * HKT:(h + 1) * HKT, :])
        return t

    mt_next = {}
    nm_next = {}

    def get_mt(b, qc, j):
        t = mt_next.get((b, qc, j))
        if t is None:
            t = mt_next[(b, qc, j)] = load_mask_pair(b, qc, j)
        return t

    def get_nm(b, qc, o):
        t = nm_next.get((b, qc, o))
        if t is None:
            t = nm_next[(b, qc, o)] = load_nm(b, qc, o)
        return t

    def get_res(b, qc, key):
        if key[0] == "mt":
            get_mt(b, qc, key[1])
        else:
            get_nm(b, qc, key[1])

    # chunk-0 startup resources come from the primed persistent tiles
    pk5, pqt, pk7, pnm0, pnm1, pmt4 = prime
    s0, s1 = ORDER_LAST[0], ORDER_LAST[1]
    kt0x = {s0: pk5, s1: pk7}
    qt_next = {(0, 0): pqt}
    nm_next[(0, 0, DIDX_OF[s0])] = pnm0
    nm_next[(0, 0, DIDX_OF[s1])] = pnm1
    mt_next[(0, 0, PAIR_OF[ORDER_LAST[2]][0])] = pmt4
    kv = {0: [load_k_half(0, 0), load_v_half(0, 0),
              load_k_half(0, 1), load_v_half(0, 1)]}

    pend = {}
    pend_pv = []

    def emit_pv(ops, pts, jj, vh0, vh1, first, final):
        vsel = vh0 if jj < HKT else vh1
        for n in range(0, QC, 512):
            nc.tensor.matmul(
                ops[:, n:n + 512],
                lhsT=vsel[:, jj % HKT, :],
                rhs=pts[jj][:, n:n + 512],
                start=first, stop=final,
                skip_group_check=True)
        del pts[jj]

    def epi_den(c):
        acc, _, b, qc = pend[c]
        den = tpsum.tile([P, NQS], fp32, name="den")
        for sq in range(NQS):
            nc.tensor.matmul(den[:, sq:sq + 1],
                             lhsT=acc[:, sq * P:(sq + 1) * P],
                             rhs=ones_col[:, :],
                             start=True, stop=True,
                             skip_group_check=True)
        rcol = outp.tile([P, NQS], fp32, name="rcol")
        nc.vector.reciprocal(out=rcol[:, :], in_=den[:, :])
        pend[c] += (rcol,)

    def epi_copy(c, last=False):
        _, ops, b, qc, _ = pend[c]
        # PSUM drain on DVE (GPSIMD cannot access PSUM on real HW); in
        # the final flush Act is already idle, so it takes half
        ot = outp.tile([P, QC], fp16, name="ot")
        if last:
            H = QC // 2
            nc.vector.tensor_copy(out=ot[:, :H], in_=ops[:, :H])
            nc.scalar.copy(out=ot[:, H:], in_=ops[:, H:])
        else:
            nc.vector.tensor_copy(out=ot[:, :], in_=ops[:, :])
        pend[c] += (ot,)

    def epi_out(c, last=False):
        _, _, b, qc, rcol, ot = pend.pop(c)
        osb = tpsum.tile([P, QC], fp16, name="osb")
        osf = outp.tile([P, NQS, D], fp16, name="osf")
        HQ = NQS // 2
        for hh in range(2):
            for t in range(hh * HQ, (hh + 1) * HQ):
                nc.tensor.transpose(osb[:, t * P:(t + 1) * P],
                                    ot[:, t * P:(t + 1) * P],
                                    ident[:, :])
            for t in range(hh * HQ, (hh + 1) * HQ):
                if last and t % 2 == 1:
                    nc.scalar.activation(
                        out=osf[:, t, :],
                        in_=osb[:, t * P:(t + 1) * P],
                        func=mybir.ActivationFunctionType.Copy,
                        scale=rcol[:, t:t + 1])
                else:
                    nc.vector.tensor_scalar_mul(
                        out=osf[:, t, :],
                        in0=osb[:, t * P:(t + 1) * P],
                        scalar1=rcol[:, t:t + 1])
            ring = (nc.sync.dma_start if (hh == 0 or last)
                    else nc.gpsimd.dma_start)
            ring(out=Od.ap()[b, qc, :, hh * HQ:(hh + 1) * HQ, :],
                 in_=osf[:, hh * HQ:(hh + 1) * HQ, :])

    for b in range(BP):
        for qc in range(NQC):
            c = b * NQC + qc
            kh0, vh0, kh1, vh1 = kv[b]
            qt = qt_next.pop((b, qc))
            if qc + 1 < NQC:
                nb, nqc = b, qc + 1
            elif b + 1 < BP:
                nb, nqc = b + 1, 0
            else:
                nb = None
            acc = accp.tile([P, QC], fp16, name="acc")
            ops = opsum.tile([P, QC], fp32, name="opsum")
            pts = {}
            last = nb is None
            order = ORDER_LAST if (last or c == 0) else list(range(NKT))
            res_plan = _res_plan(order)
            if nb is not None:
                next_last = (nb == BP - 1 and nqc == NQC - 1)
                next_plan = _res_plan(
                    ORDER_LAST if next_last else range(NKT))
            for i, kt in enumerate(order):
                sc = spsum.tile([P, QC], fp32, name="scores")
                if kt in PAIR_OF:
                    j, plane = PAIR_OF[kt]
                    mt = get_mt(b, qc, j)
                    wsel = w0 if plane == 0 else w1
                    for n in range(0, QC, 512):
                        nc.tensor.matmul(
                            sc[:, n:n + 512],
                            lhsT=wsel[:, :, :],
                            rhs=mt[:, :, n:n + 512],
                            start=True, stop=False,
                            perf_mode=mybir.MatmulPerfMode.DoubleRow,
                            skip_group_check=True)
                    qk_start = False
                else:
                    nm = get_nm(b, qc, DIDX_OF[kt])
                    qk_start = True
                if c == 0 and kt in kt0x and i < 2:
                    ksel, kloc = kt0x[kt], 0
                else:
                    ksel = kh0 if kt < HKT else kh1
                    kloc = (kt % HKT) * P
                for n in range(0, QC, 512):
                    nc.tensor.matmul(
                        sc[:, n:n + 512],
                        lhsT=ksel[:, kloc:kloc + P],
                        rhs=qt[:, n:n + 512],
                        start=qk_start, stop=True, skip_group_check=True)

                # previous chunk's PV tail + deferred epilogue, placed
                # AFTER this kt's QK so the Act pipeline never bubbles;
                # epi_copy precedes PV(c, 0) (single-buffered ops PSUM)
                if pend_pv:
                    if i == 0:
                        emit_pv(*pend_pv.pop(0))
                        emit_pv(*pend_pv.pop(0))
                    elif i == 1:
                        emit_pv(*pend_pv.pop(0))
                if c - 1 in pend:
                    if i == 1:
                        epi_den(c - 1)
                    elif i == 2:
                        epi_copy(c - 1)
                    elif i == 4:
                        epi_out(c - 1)

                pt = pp.tile([P, QC], fp16, name="pt")
                nc.scalar.activation(out=pt[:, :], in_=sc[:, :],
                                     func=Exp, scale=SCALE)
                if kt in PAIR_OF:
                    pts[kt] = pt
                else:
                    # apply the mask as an exact post-exp zeroing on DVE
                    pm = pts[kt] = pp.tile([P, QC], fp16, name="pm")
                    nc.vector.tensor_mul(out=pm[:, :], in0=pt[:, :],
                                         in1=nm[:, :])
                if i == 1:
                    nc.vector.tensor_add(out=acc[:, :],
                                         in0=pts[order[0]][:, :],
                                         in1=pts[order[1]][:, :])
                elif i > 1:
                    nc.vector.tensor_add(out=acc[:, :], in0=acc[:, :],
                                         in1=pts[kt][:, :])

                # prefetches (after compute emission so they never gate
                # it): stay ~3 k-tiles ahead in resource consumption order
                if i + 3 < NKT:
                    for key in res_plan[i + 3]:
                        get_res(b, qc, key)
                if i >= NKT - 3 and nb is not None:
                    for key in next_plan[i - (NKT - 3)]:
                        get_res(nb, nqc, key)
                if i == 6 and nb is not None:
                    qt_next[(nb, nqc)] = load_qt(nb, nqc)
                if nb is not None and nqc == 0:
                    if i == 8:
                        kv[nb] = [load_k_half(nb, 0), load_v_half(nb, 0)]
                    elif i == 10:
                        kv[nb] += [load_k_half(nb, 1), load_v_half(nb, 1)]

                # PV lags PVLAG k-tiles so the PE never waits on exp
                if i >= PVLAG:
                    emit_pv(ops, pts, order[i - PVLAG], vh0, vh1,
                            first=(i == PVLAG), final=False)
            tail = order[NKT - PVLAG:]
            for x, jj in enumerate(tail):
                pend_pv.append((ops, pts, jj, vh0, vh1, False,
                                x == len(tail) - 1))
            pend[c] = (acc, ops, b, qc)

    # final flush (no next chunk to hide it in)
    while pend_pv:
        emit_pv(*pend_pv.pop(0))
    c = BP * NQC - 1
    epi_den(c)
    epi_copy(c, last=True)
    epi_out(c, last=True)
    # software-pipelined seam: refill the chunk-0 startup tiles so the
    # next loop iteration starts compute immediately
    if load_prime is not None:
        load_prime()


def _get_nc(loop=False):
    key = f"nc_loop{loop}"
    if key not in _CACHE:
        _CACHE[key] = build_nc(loop=loop)
    return _CACHE[key]


def make_in_maps(Q, K, V, mask):
    """Host-side shard + layout prep: per-core input dicts."""
    Q = np.asarray(Q, dtype=np.float32)
    K = np.asarray(K, dtype=np.float32)
    V = np.asarray(V, dtype=np.float32)
    mask_b = np.asarray(mask).astype(bool)
    in_maps = []
    for c in range(NCORES):
        sl = slice(c * BP, (c + 1) * BP)
        qt = np.ascontiguousarray(
            Q[sl].transpose(0, 2, 1)).astype(np.float16)
        kt = np.ascontiguousarray(
            K[sl].transpose(0, 2, 1)).astype(np.float16)
        # V packed partition-major: [BP, P, NKT, D]
        v16 = np.ascontiguousarray(
            V[sl].reshape(BP, NKT, P, D).transpose(0, 2, 1, 3)
        ).astype(np.float16)
        mT = np.ascontiguousarray(mask_b[sl].transpose(0, 2, 1))
        mT4 = mT.reshape(BP, NKT, P, S)
        mt8 = np.ascontiguousarray(mT4[:, PE_TILES]).reshape(
            BP, len(PE_TILES) * P, S).astype(ml_dtypes.float8_e4m3)
        nmt = np.ascontiguousarray(~mT4[:, DVE_TILES]).reshape(
            BP, len(DVE_TILES) * P, S).astype(np.float16)
        in_maps.append({"QT": qt, "KT": kt, "V": v16, "MT": mt8,
                        "NMT": nmt})
    return in_maps


def unpack_out(raw):
    """[BP, NQC, P, NQS, D] fp16 -> [BP, S, D] fp32."""
    return np.ascontiguousarray(
        raw.transpose(0, 1, 3, 2, 4)).reshape(BP, S, D).astype(np.float32)


def kernel(Q, K, V, mask, dk=128):
    from concourse.bass_utils import run_bass_kernel_spmd

    assert int(dk) == 128
    nc = _get_nc(loop=False)
    in_maps = make_in_maps(Q, K, V, mask)
    res = run_bass_kernel_spmd(nc, in_maps, core_ids=list(range(NCORES)))
    return np.concatenate([unpack_out(r["out"]) for r in res.results],
                          axis=0)
